# revision 26
# baseline (speedup 1.0000x reference)
"""Multi-head self-attention kernel for Trainium2, batch-parallel over 8 NeuronCores.

Problem: B=8, S=1024, IN_DIM=D_MODEL=768, H=12, DK=64.
  q/k/v = Q @ W{q,k,v}.T + b   -> [b, H, s, dk]
  scores = exp(q k^T / 8) * key_mask ; attn = scores / sum
  out = attn @ v -> [b, s, 768]

Strategy (per core = one batch element):
  - host: QT = Q[b].T, WT = W.T (m-chunked), maskbias[p, t] = 0 / -60 from length
  - v in [s, d] layout with a ones-column appended per head (rowsum trick)
  - qT/kT in [d, s] layout, per head-pair (d-tile)
  - scoresT[sk, sq] via K=64 matmuls (two heads packed in PE rows 0-63 / 64-127)
  - exp fused with mask bias + 1/sqrt(dk) scale on ACT, writes bf16 probsT
  - ctx chains packed 7-per-PSUM-bank; batched normalize: one strided
    reciprocal per bank + stride-0-broadcast tensor_tensor multiplies
  - output DMA'd per head-pair column strip (overlapped with compute)
  - software pipeline: ctx of pair t-1 interleaved with scores of pair t
"""

import functools
import sys
import types

import numpy as np

B, S, IN_DIM, D_MODEL, H = 8, 1024, 768, 768, 12
DK = D_MODEL // H
NCORES = 8
NKT = IN_DIM // 128   # 6 contraction tiles
NDT = D_MODEL // 128  # 6 d-tiles (head pairs)
NST = S // 128        # 8 s-tiles
# Masking is fully multiplicative: masked kT columns and masked v rows (and
# their rowsum-ones entries) are exact zeros, so masked keys contribute 0 to
# both the context numerator and denominator regardless of what exp() yields
# on their (zeroed) scores. This also slashes PE switching power on
# short-length cores (the chip throttles the clock once its activity window
# expires, so total switching energy matters fleet-wide).


def _install_shims():
    """antenv.axon_hooks shim (for NTFF tracing) + Tile drain-wait splitting
    (this walrus build accepts only one sync-wait command per Drain/CTRL)."""
    if 'antenv.axon_hooks' not in sys.modules:
        mod = types.ModuleType('antenv.axon_hooks')
        mod._hook = None
        mod.set_axon_ntff_profile_hook = lambda h: setattr(mod, '_hook', h)
        mod.get_axon_ntff_profile_hook = lambda: mod._hook
        sys.modules['antenv.axon_hooks'] = mod
        try:
            import antenv
            antenv.axon_hooks = mod
            from trn_agent_boot.trn_boot import _ntff_profile_via_ctypes
            mod.set_axon_ntff_profile_hook(
                _ntff_profile_via_ctypes('/opt/axon/libaxon_pjrt.so'))
        except Exception:
            pass

    import concourse.tile as tile
    if getattr(tile.TileContext, '_drain_patched', False):
        return
    from concourse.vector_clock import ScopedClock, VectorClock

    def _patched_drain_and_barrier(self, tick_clock, wait_clock):
        nc = self.nc
        gvec = tick_clock.global_clock
        n = len(gvec)
        for i in range(n):
            t = gvec[i]
            if t <= 0:
                continue
            v = [0] * n
            v[i] = t
            nop = nc.sync.nop(nofuse=True, hint="drain_wait_split")
            wait_clock.add_sem_waits(nop.ins, ScopedClock({None: VectorClock(v)}))
        # The per-proc NOPs above carry every wait (SP queue is in-order),
        # so the drain itself needs none.
        nc.sync.drain()
        nc.all_engine_barrier()
        assert self.sems is not None
        popped = nc._tile_sem_poison_stack.pop()
        assert popped is self._sem_poison
        nc.clear_and_free_semaphores(list(self.sems.allocated().values()))
        nc.all_engine_barrier()

    tile.TileContext._drain_and_barrier = _patched_drain_and_barrier

    # This walrus build accepts at most ONE sync-wait command per engine
    # instruction: split extra waits onto non-fusable NOPs emitted just
    # before the instruction on the same engine queue.
    import bass_rust
    import concourse.mybir as mybir
    _orig_lower = tile.TileContext._lower_ordered_insts

    def _split_waits_then_lower(self, ordered):
        nc = self.nc
        for bbname, insts in ordered.items():
            need = any(
                i.sync_info is not None and i.sync_info.on_wait
                and len(i.sync_info.on_wait) > 1
                for i in insts)
            if not need:
                continue
            out = []
            for inst in insts:
                si = inst.sync_info
                if si is not None and si.on_wait and len(si.on_wait) > 1:
                    waits = list(si.on_wait)
                    for w in waits[:-1]:
                        nop = mybir.InstNoOp(
                            name=nc.get_next_instruction_name(), ins=[], outs=[])
                        nop.engine = inst.engine
                        nop.bass_nofuse = True
                        nop.sync_info = bass_rust.SyncInfo(
                            on_wait=[w], on_update=[])
                        out.append(nop)
                    inst.sync_info = bass_rust.SyncInfo(
                        on_wait=[waits[-1]],
                        on_update=list(si.on_update or []))
                out.append(inst)
            insts[:] = out
        return _orig_lower(self, ordered)

    tile.TileContext._lower_ordered_insts = _split_waits_then_lower
    tile.TileContext._drain_patched = True


@functools.lru_cache(maxsize=None)
def _build_program(n_sk: int, use_bias: bool):
    import concourse.bass as bass
    import concourse.tile as tile
    import concourse.mybir as mybir
    from contextlib import ExitStack

    f32 = mybir.dt.float32
    bf16 = mybir.dt.bfloat16
    EXP = mybir.ActivationFunctionType.Exp
    MULT = mybir.AluOpType.mult

    nc = bass.Bass("TRN2", enable_partition_id=False)
    qt_d = nc.dram_tensor("qt", (IN_DIM, S), bf16, kind="ExternalInput")
    wqm_d = nc.dram_tensor("wqm", (NDT, IN_DIM, 128), bf16, kind="ExternalInput")
    wkm_d = nc.dram_tensor("wkm", (NDT, IN_DIM, 128), bf16, kind="ExternalInput")
    wvt_d = nc.dram_tensor("wvt", (IN_DIM, D_MODEL), bf16, kind="ExternalInput")
    # mask bias (cols 0:NST) and 0/1 key mask (cols NST:2*NST), one DMA
    mbv_d = nc.dram_tensor("mbv", (128, 2 * NST), f32, kind="ExternalInput")
    km_d = nc.dram_tensor("km", (128, S), bf16, kind="ExternalInput")
    if use_bias:
        bq_d = nc.dram_tensor("bq", (1, D_MODEL), bf16, kind="ExternalInput")
        bk_d = nc.dram_tensor("bk", (1, D_MODEL), bf16, kind="ExternalInput")
        bv_d = nc.dram_tensor("bv", (1, D_MODEL), bf16, kind="ExternalInput")
    out_d = nc.dram_tensor("out", (S, D_MODEL), f32, kind="ExternalOutput")

    with tile.TileContext(nc) as tc, ExitStack() as ctx:
        const = ctx.enter_context(tc.tile_pool(name="const", bufs=1))
        big = ctx.enter_context(tc.tile_pool(name="big", bufs=1))
        wpool = ctx.enter_context(tc.tile_pool(name="w", bufs=3))
        qkpool = ctx.enter_context(tc.tile_pool(name="qk", bufs=3))
        prpool = ctx.enter_context(tc.tile_pool(name="pr", bufs=1))
        smpool = ctx.enter_context(tc.tile_pool(name="sm", bufs=6))
        pj = ctx.enter_context(tc.tile_pool(name="pj", bufs=2, space="PSUM"))
        sc = ctx.enter_context(tc.tile_pool(name="sc", bufs=1, space="PSUM"))
        cx = ctx.enter_context(tc.tile_pool(name="cx", bufs=1, space="PSUM"))

        # ---- input DMAs. Priority: the first qkproj matmul needs wq0 + qt0,
        # so those go first on their queues; masks and wvt trail (first exp
        # is ~7us in, vproj starts mid-pair-0).
        wqk_pending = {}

        def prefetch_wqk(t):
            wq_sb = wpool.tile([128, NKT, 128], bf16, tag="wq", name=f"wq{t}")
            nc.sync.dma_start(
                out=wq_sb, in_=wqm_d[t].rearrange("(k p) m -> p k m", p=128))
            wk_sb = wpool.tile([128, NKT, 128], bf16, tag="wk", name=f"wk{t}")
            nc.sync.dma_start(
                out=wk_sb, in_=wkm_d[t].rearrange("(k p) m -> p k m", p=128))
            wqk_pending[t] = (wq_sb, wk_sb)

        qt_sb = []
        qt_tiles = []
        for k in range(NKT):
            qk_t = big.tile([128, S], bf16, name=f"qtsb{k}")
            qt_sb.append(qk_t)
        qt_engs = [nc.gpsimd, nc.scalar, nc.gpsimd, nc.scalar, nc.gpsimd, nc.scalar]
        qt_engs[0].dma_start(out=qt_sb[0], in_=qt_d[0:128, :])
        prefetch_wqk(0)
        for k in range(1, NKT):
            qt_engs[k].dma_start(
                out=qt_sb[k], in_=qt_d[k * 128:(k + 1) * 128, :])
        mbv_sb = const.tile([128, 2 * NST], f32)
        nc.gpsimd.dma_start(out=mbv_sb, in_=mbv_d[:, :])
        mb_sb = mbv_sb[:, 0:NST]
        mv_sb = mbv_sb[:, NST:2 * NST]
        km_sb = const.tile([128, S], bf16)
        nc.scalar.dma_start(out=km_sb, in_=km_d[:, :])
        wvt_sb = []
        wvt_engs = [nc.gpsimd, nc.sync, nc.scalar, nc.gpsimd, nc.sync, nc.scalar]
        for k in range(NKT):
            wv_t = big.tile([128, D_MODEL], bf16, name=f"wvtsb{k}")
            wvt_engs[k].dma_start(out=wv_t, in_=wvt_d[k * 128:(k + 1) * 128, :])
            wvt_sb.append(wv_t)
        v_sb = big.tile([128, NST, H * (DK + 1)], bf16)
        out_sb = big.tile([128, NST, D_MODEL], f32)
        if use_bias:
            ones_sb = const.tile([1, 512], bf16)
            nc.vector.memset(ones_sb, 1.0)
            bq_sb = const.tile([1, D_MODEL], bf16)
            nc.sync.dma_start(out=bq_sb, in_=bq_d[:, :])
            bk_sb = const.tile([1, D_MODEL], bf16)
            nc.sync.dma_start(out=bk_sb, in_=bk_d[:, :])
            bv_sb = const.tile([1, D_MODEL], bf16)
            nc.sync.dma_start(out=bv_sb, in_=bv_d[:, :])

        # ---- v projection: [s, d] layout, heads strided by 65 with ones col
        def emit_vproj(srow):
            for nch in range(2):  # 384 cols = 6 heads each
                ps = pj.tile([128, 384], f32, tag="px", name=f"psv{srow}_{nch}")
                for k in range(NKT):
                    nc.tensor.matmul(
                        ps,
                        lhsT=qt_sb[k][:, srow * 128:(srow + 1) * 128],
                        rhs=wvt_sb[k][:, nch * 384:(nch + 1) * 384],
                        start=(k == 0), stop=(k == NKT - 1 and not use_bias))
                if use_bias:
                    nc.tensor.matmul(
                        ps, lhsT=ones_sb[0:1, 0:128],
                        rhs=bv_sb[0:1, nch * 384:(nch + 1) * 384],
                        start=False, stop=True)
                dst = v_sb[:, srow, nch * 390:(nch + 1) * 390]
                dst3 = dst.rearrange("p (h x) -> p h x", x=DK + 1)[:, :, 0:DK]
                src3 = ps.rearrange("p (h x) -> p h x", x=DK)
                # zero masked-key rows (per-partition mask)
                nc.vector.tensor_scalar_mul(dst3, src3, mv_sb[:, srow:srow + 1])
            # the rowsum ones-column is the key mask itself
            ones_dst = v_sb[:, srow, :].rearrange(
                "p (h x) -> p h x", x=DK + 1)[:, :, DK:DK + 1]
            nc.vector.tensor_copy(
                out=ones_dst,
                in_=mv_sb[:, srow:srow + 1].unsqueeze(-1).to_broadcast([128, H, 1]))

        # ---- per head-pair machinery.
        # qkproj is emitted in 4 chunk-sized pieces so it can interleave
        # between scores emissions (the single-buffered scores psum needs
        # ~1.2us of other PE work between consecutive sk's to hide the exp).
        qk_tiles = {}

        def start_qkproj(t):
            qT = qkpool.tile([128, S], bf16, tag="qT", name=f"qT{t}")
            kT = qkpool.tile([128, S], bf16, tag="kT", name=f"kT{t}")
            qk_tiles[t] = (qT, kT) + wqk_pending.pop(t)

        def emit_qkproj_chunk(t, chunk):
            qT, kT, wq_sb, wk_sb = qk_tiles[t]
            ti, nch = divmod(chunk, 2)
            w_sb = (wq_sb, wk_sb)[ti]
            dstT = (qT, kT)[ti]
            ps = pj.tile([128, 512], f32, tag="px", name=f"psqk{t}_{chunk}")
            for k in range(NKT):
                nc.tensor.matmul(
                    ps,
                    lhsT=w_sb[:, k, :],
                    rhs=qt_sb[k][:, nch * 512:(nch + 1) * 512],
                    start=(k == 0), stop=(k == NKT - 1 and not use_bias))
            if use_bias:
                bias_sb = bq_sb if ti == 0 else bk_sb
                nc.tensor.matmul(
                    ps,
                    lhsT=bias_sb[0:1, t * 128:(t + 1) * 128],
                    rhs=ones_sb[0:1, 0:512],
                    start=False, stop=True)
            if ti == 1:
                # km carries the key mask AND the 1/sqrt(dk) scale: masked
                # kT columns become exact zeros (kills PE switching power in
                # the scores matmuls of short-length cores) and the exp
                # needs no scale/bias, so one ACT call covers both heads.
                nc.vector.tensor_tensor(
                    out=dstT[:, nch * 512:(nch + 1) * 512], in0=ps,
                    in1=km_sb[:, nch * 512:(nch + 1) * 512],
                    op=mybir.AluOpType.mult)
            else:
                nc.vector.tensor_copy(
                    out=dstT[:, nch * 512:(nch + 1) * 512], in_=ps)

        probs = {}

        def emit_scores_sk(t, sk):
            qT, kT = qk_tiles[t][:2]
            # one 4-bank psum tile per sk: [hl0 | hl1] x 1024 queries.
            # kT carries the 1/sqrt(dk) scale and the key mask (km), so the
            # exp is pure: a single ACT instruction covers both heads.
            pss = sc.tile([128, 2 * S], f32, tag="sc", name=f"sc{t}_{sk}")
            for hl in range(2):
                lo, hi = hl * 64, (hl + 1) * 64
                for nch in range(2):
                    nc.tensor.matmul(
                        pss[:, hl * S + nch * 512:hl * S + (nch + 1) * 512],
                        lhsT=kT[lo:hi, sk * 128:(sk + 1) * 128],
                        rhs=qT[lo:hi, nch * 512:(nch + 1) * 512],
                        start=True, stop=True)
            pb = prpool.tile([128, 2 * S], bf16, tag=f"pb{t % 2}_{sk}",
                             name=f"pb{t}_{sk}")
            nc.scalar.activation(out=pb, in_=pss, func=EXP,
                                 bias=mb_sb[:, 0:1])
            probs[(t % 2, sk)] = pb

        # ctx batches: per pair, 16 chains (hl, sq) of 65 psum cols each,
        # packed 7 + 7 + 2 into three bank-tiles (tags cxA, cxB, cxA).
        CHAINS = [(hl, sq) for hl in range(2) for sq in range(NST)]
        BATCHES = [(0, 7, "cxA"), (7, 14, "cxB"), (14, 16, "cxA")]

        def emit_ctx_batch(t, bi):
            c0, c1, tag = BATCHES[bi]
            nch = c1 - c0
            pc = cx.tile([128, nch * (DK + 1)], f32, tag=tag,
                         name=f"cx{t}_{bi}")
            for ci in range(nch):
                hl, sq = CHAINS[c0 + ci]
                head = 2 * t + hl
                for sk in range(n_sk):
                    nc.tensor.matmul(
                        pc[:, ci * (DK + 1):ci * (DK + 1) + DK + 1],
                        lhsT=probs[(t % 2, sk)][:, hl * S + sq * 128:
                                                hl * S + (sq + 1) * 128],
                        rhs=v_sb[:, sk, head * (DK + 1):(head + 1) * (DK + 1)],
                        start=(sk == 0), stop=(sk == n_sk - 1))
            return pc

        def emit_ctx_normalize(t, bi, pc):
            c0, c1, tag = BATCHES[bi]
            nch = c1 - c0
            pc3 = pc.rearrange("p (c x) -> p c x", x=DK + 1)
            rec = smpool.tile([128, nch], f32, name=f"rec{t}_{bi}")
            nc.vector.reciprocal(rec, pc3[:, :, DK])
            # runs of consecutive chains with the same head
            ci = 0
            while ci < nch:
                hl0 = CHAINS[c0 + ci][0]
                cj = ci
                while cj < nch and CHAINS[c0 + cj][0] == hl0:
                    cj += 1
                cnt = cj - ci
                head = 2 * t + hl0
                sq0 = CHAINS[c0 + ci][1]
                out_ap = out_sb[:, sq0:sq0 + cnt, head * DK:(head + 1) * DK]
                in0 = pc3[:, ci:cj, 0:DK]
                in1 = rec[:, ci:cj].unsqueeze(-1).to_broadcast([128, cnt, DK])
                nc.vector.tensor_tensor(out=out_ap, in0=in0, in1=in1, op=MULT)
                ci = cj

        def emit_out_strip(t):
            # pair t's 128 output columns for all 1024 rows, 2 DMAs
            for half, eng in ((0, nc.gpsimd), (1, nc.sync)):
                rows = out_d[half * 512:(half + 1) * 512,
                             t * 128:(t + 1) * 128]
                eng.dma_start(
                    out=rows.rearrange("(s p) c -> p s c", p=128),
                    in_=out_sb[:, half * 4:(half + 1) * 4,
                               t * 128:(t + 1) * 128])

        def emit_out_batch(t, bi, eng):
            # DMA just the chains normalized in batch bi (tail overlap)
            c0, c1, _ = BATCHES[bi]
            ci = c0
            while ci < c1:
                hl0, sq0 = CHAINS[ci]
                cj = ci
                while cj < c1 and CHAINS[cj][0] == hl0:
                    cj += 1
                cnt = cj - ci
                head = 2 * t + hl0
                rows = out_d[sq0 * 128:(sq0 + cnt) * 128,
                             head * DK:(head + 1) * DK]
                eng.dma_start(
                    out=rows.rearrange("(s p) c -> p s c", p=128),
                    in_=out_sb[:, sq0:sq0 + cnt, head * DK:(head + 1) * DK])
                ci = cj

        # ---- main pipeline
        start_qkproj(0)
        for c in range(4):
            emit_qkproj_chunk(0, c)
        prefetch_wqk(1)
        vi = 0
        for sk in range(n_sk):
            emit_scores_sk(0, sk)
            while vi < NST * (sk + 1) // n_sk:
                emit_vproj(vi)
                vi += 1
        while vi < NST:
            emit_vproj(vi)
            vi += 1
        start_qkproj(1)
        for c in range(4):
            emit_qkproj_chunk(1, c)

        for t in range(1, NDT):
            last = (t + 1 >= NDT)
            if not last:
                prefetch_wqk(t + 1)
            pcs = {}

            def run_action(ai, t=t):
                if ai == 0:
                    pcs[0] = emit_ctx_batch(t - 1, 0)
                elif ai == 1:
                    emit_ctx_normalize(t - 1, 0, pcs[0])
                    pcs[1] = emit_ctx_batch(t - 1, 1)
                elif ai == 2:
                    emit_ctx_normalize(t - 1, 1, pcs[1])
                    pcs[2] = emit_ctx_batch(t - 1, 2)
                else:
                    emit_ctx_normalize(t - 1, 2, pcs[2])
                    emit_out_strip(t - 1)

            thresholds = [3, 4, 5, 6] if last else [4, 5, 6, 7]
            ai = 0
            for sk in range(n_sk):
                emit_scores_sk(t, sk)
                if not last and sk < 4:
                    if sk == 0:
                        start_qkproj(t + 1)
                    emit_qkproj_chunk(t + 1, sk)
                while ai < 4 and sk >= min(thresholds[ai], n_sk - 1):
                    run_action(ai)
                    ai += 1
            if not last:
                for c in range(min(4, n_sk), 4):
                    emit_qkproj_chunk(t + 1, c)
            while ai < 4:
                run_action(ai)
                ai += 1

        # epilogue: ctx of the last pair; DMA each batch as it normalizes
        t = NDT - 1
        pc0 = emit_ctx_batch(t, 0)
        pc1 = emit_ctx_batch(t, 1)
        emit_ctx_normalize(t, 0, pc0)
        emit_out_batch(t, 0, nc.gpsimd)
        pc2 = emit_ctx_batch(t, 2)
        emit_ctx_normalize(t, 1, pc1)
        emit_out_batch(t, 1, nc.sync)
        emit_ctx_normalize(t, 2, pc2)
        emit_out_batch(t, 2, nc.gpsimd)

    return nc


TRACE = False
LAST_EXEC_NS = None
LAST_RES = None


def kernel(Q, length, Wq, bq, Wk, bk, Wv, bv):
    global LAST_EXEC_NS, LAST_RES
    _install_shims()
    from concourse.bass_utils import run_bass_kernel_spmd

    Q = np.asarray(Q, np.float32)
    length = np.asarray(length, np.int32)
    Wq, Wk, Wv = (np.asarray(w, np.float32) for w in (Wq, Wk, Wv))
    bq, bk, bv = (np.asarray(b, np.float32) for b in (bq, bk, bv))

    use_bias = bool(np.any(bq) or np.any(bk) or np.any(bv))
    maxlen = int(length.max()) if length.size else S
    n_sk = max(1, min(NST, -(-max(1, maxlen) // 128)))

    import ml_dtypes
    bfl = ml_dtypes.bfloat16
    qt_all = np.ascontiguousarray(Q.transpose(0, 2, 1)).astype(bfl)   # [B, 768, 1024]
    wqm = np.ascontiguousarray(Wq.T.reshape(IN_DIM, NDT, 128).transpose(1, 0, 2)).astype(bfl)
    wkm = np.ascontiguousarray(Wk.T.reshape(IN_DIM, NDT, 128).transpose(1, 0, 2)).astype(bfl)
    wvt = np.ascontiguousarray(Wv.T).astype(bfl)                      # [768, 768]
    j = np.arange(S)
    valid = j[None, :] < length[:, None]                                   # [B, S]
    # mb half is the (zero) exp bias AP; mv half is the 0/1 key-row mask
    mb = np.zeros((B, 128, NST), np.float32)
    mv = valid.astype(np.float32).reshape(B, NST, 128).transpose(0, 2, 1)
    mbv = np.ascontiguousarray(np.concatenate([mb, mv], axis=2))           # [B,128,16]
    # km folds the key mask and the 1/sqrt(dk) scale into kT
    kmrow = np.where(valid, np.float32(1.0 / np.sqrt(DK)), np.float32(0.0))
    km = np.ascontiguousarray(
        np.broadcast_to(kmrow.astype(bfl)[:, None, :], (B, 128, S)))       # [B,128,S]

    nc = _build_program(n_sk, use_bias)
    in_maps = []
    for b in range(B):
        m = {"qt": qt_all[b], "wqm": wqm, "wkm": wkm, "wvt": wvt,
             "mbv": mbv[b], "km": km[b]}
        if use_bias:
            m["bq"] = bq.reshape(1, -1).astype(np.float32).astype(bfl)
            m["bk"] = bk.reshape(1, -1).astype(np.float32).astype(bfl)
            m["bv"] = bv.reshape(1, -1).astype(np.float32).astype(bfl)
        in_maps.append(m)

    res = run_bass_kernel_spmd(
        nc, in_maps, core_ids=list(range(NCORES)), trace=TRACE)
    LAST_EXEC_NS = res.exec_time_ns
    LAST_RES = res
    out = np.stack([res.results[b]["out"] for b in range(B)])
    out = np.ascontiguousarray(out.astype(np.float32))
    # reference: attn = p / (sum + 1e-8); for length==0 every key is masked
    # and the reference output is ~0, while our reciprocal normalization
    # averages the tiny masked probs. Zero those rows host-side.
    for b in range(B):
        if int(length[b]) == 0:
            out[b] = 0.0
    return out


# revision 29
# speedup vs baseline: 1.1438x; 1.1438x over previous
"""Multi-head self-attention kernel for Trainium2, batch-parallel over 8 NeuronCores.

Problem: B=8, S=1024, IN_DIM=D_MODEL=768, H=12, DK=64.
  q/k/v = Q @ W{q,k,v}.T + b   -> [b, H, s, dk]
  scores = exp(q k^T / 8) * key_mask ; attn = scores / sum
  out = attn @ v -> [b, s, 768]

Strategy (per core = one batch element):
  - host: QT = Q[b].T, WT = W.T (m-chunked), maskbias[p, t] = 0 / -60 from length
  - v in [s, d] layout with a ones-column appended per head (rowsum trick)
  - qT/kT in [d, s] layout, per head-pair (d-tile)
  - scoresT[sk, sq] via K=64 matmuls (two heads packed in PE rows 0-63 / 64-127)
  - exp fused with mask bias + 1/sqrt(dk) scale on ACT, writes bf16 probsT
  - ctx chains packed 7-per-PSUM-bank; batched normalize: one strided
    reciprocal per bank + stride-0-broadcast tensor_tensor multiplies
  - output DMA'd per head-pair column strip (overlapped with compute)
  - software pipeline: ctx of pair t-1 interleaved with scores of pair t
"""

import functools
import sys
import types

import numpy as np

B, S, IN_DIM, D_MODEL, H = 8, 1024, 768, 768, 12
DK = D_MODEL // H
NCORES = 8
NKT = IN_DIM // 128   # 6 contraction tiles
NDT = D_MODEL // 128  # 6 d-tiles (head pairs)
NST = S // 128        # 8 s-tiles
# Masking is fully multiplicative: masked kT columns and masked v rows (and
# their rowsum-ones entries) are exact zeros, so masked keys contribute 0 to
# both the context numerator and denominator regardless of what exp() yields
# on their (zeroed) scores. This also slashes PE switching power on
# short-length cores (the chip throttles the clock once its activity window
# expires, so total switching energy matters fleet-wide).


def _install_shims():
    """antenv.axon_hooks shim (for NTFF tracing) + Tile drain-wait splitting
    (this walrus build accepts only one sync-wait command per Drain/CTRL)."""
    if 'antenv.axon_hooks' not in sys.modules:
        mod = types.ModuleType('antenv.axon_hooks')
        mod._hook = None
        mod.set_axon_ntff_profile_hook = lambda h: setattr(mod, '_hook', h)
        mod.get_axon_ntff_profile_hook = lambda: mod._hook
        sys.modules['antenv.axon_hooks'] = mod
        try:
            import antenv
            antenv.axon_hooks = mod
            from trn_agent_boot.trn_boot import _ntff_profile_via_ctypes
            mod.set_axon_ntff_profile_hook(
                _ntff_profile_via_ctypes('/opt/axon/libaxon_pjrt.so'))
        except Exception:
            pass

    import concourse.tile as tile
    if getattr(tile.TileContext, '_drain_patched', False):
        return
    from concourse.vector_clock import ScopedClock, VectorClock

    def _patched_drain_and_barrier(self, tick_clock, wait_clock):
        nc = self.nc
        gvec = tick_clock.global_clock
        n = len(gvec)
        for i in range(n):
            t = gvec[i]
            if t <= 0:
                continue
            v = [0] * n
            v[i] = t
            nop = nc.sync.nop(nofuse=True, hint="drain_wait_split")
            wait_clock.add_sem_waits(nop.ins, ScopedClock({None: VectorClock(v)}))
        # The per-proc NOPs above carry every wait (SP queue is in-order),
        # so the drain itself needs none.
        nc.sync.drain()
        nc.all_engine_barrier()
        assert self.sems is not None
        popped = nc._tile_sem_poison_stack.pop()
        assert popped is self._sem_poison
        nc.clear_and_free_semaphores(list(self.sems.allocated().values()))
        nc.all_engine_barrier()

    tile.TileContext._drain_and_barrier = _patched_drain_and_barrier

    # This walrus build accepts at most ONE sync-wait command per engine
    # instruction: split extra waits onto non-fusable NOPs emitted just
    # before the instruction on the same engine queue.
    import bass_rust
    import concourse.mybir as mybir
    _orig_lower = tile.TileContext._lower_ordered_insts

    def _split_waits_then_lower(self, ordered):
        nc = self.nc
        for bbname, insts in ordered.items():
            need = any(
                i.sync_info is not None and i.sync_info.on_wait
                and len(i.sync_info.on_wait) > 1
                for i in insts)
            if not need:
                continue
            out = []
            for inst in insts:
                si = inst.sync_info
                if si is not None and si.on_wait and len(si.on_wait) > 1:
                    waits = list(si.on_wait)
                    for w in waits[:-1]:
                        nop = mybir.InstNoOp(
                            name=nc.get_next_instruction_name(), ins=[], outs=[])
                        nop.engine = inst.engine
                        nop.bass_nofuse = True
                        nop.sync_info = bass_rust.SyncInfo(
                            on_wait=[w], on_update=[])
                        out.append(nop)
                    inst.sync_info = bass_rust.SyncInfo(
                        on_wait=[waits[-1]],
                        on_update=list(si.on_update or []))
                out.append(inst)
            insts[:] = out
        return _orig_lower(self, ordered)

    tile.TileContext._lower_ordered_insts = _split_waits_then_lower
    tile.TileContext._drain_patched = True


@functools.lru_cache(maxsize=None)
def _build_program(n_sk: int, use_bias: bool):
    import concourse.bass as bass
    import concourse.tile as tile
    import concourse.mybir as mybir
    from contextlib import ExitStack

    f32 = mybir.dt.float32
    bf16 = mybir.dt.bfloat16
    EXP = mybir.ActivationFunctionType.Exp
    MULT = mybir.AluOpType.mult

    nc = bass.Bass("TRN2", enable_partition_id=False)
    qt_d = nc.dram_tensor("qt", (IN_DIM, S), bf16, kind="ExternalInput")
    wqm_d = nc.dram_tensor("wqm", (NDT, IN_DIM, 128), bf16, kind="ExternalInput")
    wkm_d = nc.dram_tensor("wkm", (NDT, IN_DIM, 128), bf16, kind="ExternalInput")
    wvt_d = nc.dram_tensor("wvt", (IN_DIM, D_MODEL), bf16, kind="ExternalInput")
    # mask bias (cols 0:NST) and 0/1 key mask (cols NST:2*NST), one DMA
    mbv_d = nc.dram_tensor("mbv", (128, 2 * NST), f32, kind="ExternalInput")
    km_d = nc.dram_tensor("km", (128, S), bf16, kind="ExternalInput")
    if use_bias:
        bq_d = nc.dram_tensor("bq", (1, D_MODEL), bf16, kind="ExternalInput")
        bk_d = nc.dram_tensor("bk", (1, D_MODEL), bf16, kind="ExternalInput")
        bv_d = nc.dram_tensor("bv", (1, D_MODEL), bf16, kind="ExternalInput")
    out_d = nc.dram_tensor("out", (S, D_MODEL), f32, kind="ExternalOutput")

    with tile.TileContext(nc) as tc, ExitStack() as ctx:
        const = ctx.enter_context(tc.tile_pool(name="const", bufs=1))
        big = ctx.enter_context(tc.tile_pool(name="big", bufs=1))
        wpool = ctx.enter_context(tc.tile_pool(name="w", bufs=3))
        qkpool = ctx.enter_context(tc.tile_pool(name="qk", bufs=3))
        prpool = ctx.enter_context(tc.tile_pool(name="pr", bufs=1))
        smpool = ctx.enter_context(tc.tile_pool(name="sm", bufs=6))
        pj = ctx.enter_context(tc.tile_pool(name="pj", bufs=2, space="PSUM"))
        sc = ctx.enter_context(tc.tile_pool(name="sc", bufs=2, space="PSUM"))
        cx = ctx.enter_context(tc.tile_pool(name="cx", bufs=1, space="PSUM"))

        # ---- input DMAs. Priority: the first qkproj matmul needs wq0 + qt0,
        # so those go first on their queues; masks and wvt trail (first exp
        # is ~7us in, vproj starts mid-pair-0).
        wqk_pending = {}

        def prefetch_wqk(t):
            wq_sb = wpool.tile([128, NKT, 128], bf16, tag="wq", name=f"wq{t}")
            nc.sync.dma_start(
                out=wq_sb, in_=wqm_d[t].rearrange("(k p) m -> p k m", p=128))
            wk_sb = wpool.tile([128, NKT, 128], bf16, tag="wk", name=f"wk{t}")
            nc.sync.dma_start(
                out=wk_sb, in_=wkm_d[t].rearrange("(k p) m -> p k m", p=128))
            wqk_pending[t] = (wq_sb, wk_sb)

        qt_sb = []
        qt_tiles = []
        for k in range(NKT):
            qk_t = big.tile([128, S], bf16, name=f"qtsb{k}")
            qt_sb.append(qk_t)
        qt_engs = [nc.gpsimd, nc.scalar, nc.gpsimd, nc.scalar, nc.gpsimd, nc.scalar]
        qt_engs[0].dma_start(out=qt_sb[0], in_=qt_d[0:128, :])
        prefetch_wqk(0)
        for k in range(1, NKT):
            qt_engs[k].dma_start(
                out=qt_sb[k], in_=qt_d[k * 128:(k + 1) * 128, :])
        mbv_sb = const.tile([128, 2 * NST], f32)
        nc.gpsimd.dma_start(out=mbv_sb, in_=mbv_d[:, :])
        mb_sb = mbv_sb[:, 0:NST]
        mv_sb = mbv_sb[:, NST:2 * NST]
        km_sb = const.tile([128, S], bf16)
        nc.scalar.dma_start(out=km_sb, in_=km_d[:, :])
        wvt_sb = []
        wvt_engs = [nc.gpsimd, nc.sync, nc.scalar, nc.gpsimd, nc.sync, nc.scalar]
        for k in range(NKT):
            wv_t = big.tile([128, D_MODEL], bf16, name=f"wvtsb{k}")
            wvt_engs[k].dma_start(out=wv_t, in_=wvt_d[k * 128:(k + 1) * 128, :])
            wvt_sb.append(wv_t)
        v_sb = big.tile([128, NST, H * (DK + 1)], bf16)
        out_sb = big.tile([128, NST, D_MODEL], f32)
        if use_bias:
            ones_sb = const.tile([1, 512], bf16)
            nc.vector.memset(ones_sb, 1.0)
            bq_sb = const.tile([1, D_MODEL], bf16)
            nc.sync.dma_start(out=bq_sb, in_=bq_d[:, :])
            bk_sb = const.tile([1, D_MODEL], bf16)
            nc.sync.dma_start(out=bk_sb, in_=bk_d[:, :])
            bv_sb = const.tile([1, D_MODEL], bf16)
            nc.sync.dma_start(out=bv_sb, in_=bv_d[:, :])

        # ---- v projection: [s, d] layout, heads strided by 65 with ones col
        def emit_vproj(srow):
            for nch in range(2):  # 384 cols = 6 heads each
                ps = pj.tile([128, 384], f32, tag="px", name=f"psv{srow}_{nch}")
                for k in range(NKT):
                    nc.tensor.matmul(
                        ps,
                        lhsT=qt_sb[k][:, srow * 128:(srow + 1) * 128],
                        rhs=wvt_sb[k][:, nch * 384:(nch + 1) * 384],
                        start=(k == 0), stop=(k == NKT - 1 and not use_bias))
                if use_bias:
                    nc.tensor.matmul(
                        ps, lhsT=ones_sb[0:1, 0:128],
                        rhs=bv_sb[0:1, nch * 384:(nch + 1) * 384],
                        start=False, stop=True)
                dst = v_sb[:, srow, nch * 390:(nch + 1) * 390]
                dst3 = dst.rearrange("p (h x) -> p h x", x=DK + 1)[:, :, 0:DK]
                src3 = ps.rearrange("p (h x) -> p h x", x=DK)
                # zero masked-key rows (per-partition mask)
                nc.vector.tensor_scalar_mul(dst3, src3, mv_sb[:, srow:srow + 1])
            # the rowsum ones-column is the key mask itself
            ones_dst = v_sb[:, srow, :].rearrange(
                "p (h x) -> p h x", x=DK + 1)[:, :, DK:DK + 1]
            nc.vector.tensor_copy(
                out=ones_dst,
                in_=mv_sb[:, srow:srow + 1].unsqueeze(-1).to_broadcast([128, H, 1]))

        # ---- per head-pair machinery.
        # qkproj is emitted in 4 chunk-sized pieces so it can interleave
        # between scores emissions (the single-buffered scores psum needs
        # ~1.2us of other PE work between consecutive sk's to hide the exp).
        qk_tiles = {}

        def start_qkproj(t):
            qT = qkpool.tile([128, S], bf16, tag="qT", name=f"qT{t}")
            kT = qkpool.tile([128, S], bf16, tag="kT", name=f"kT{t}")
            qk_tiles[t] = (qT, kT) + wqk_pending.pop(t)

        def emit_qkproj_chunk(t, chunk):
            qT, kT, wq_sb, wk_sb = qk_tiles[t]
            ti, nch = divmod(chunk, 2)
            w_sb = (wq_sb, wk_sb)[ti]
            dstT = (qT, kT)[ti]
            ps = pj.tile([128, 512], f32, tag="px", name=f"psqk{t}_{chunk}")
            for k in range(NKT):
                nc.tensor.matmul(
                    ps,
                    lhsT=w_sb[:, k, :],
                    rhs=qt_sb[k][:, nch * 512:(nch + 1) * 512],
                    start=(k == 0), stop=(k == NKT - 1 and not use_bias))
            if use_bias:
                bias_sb = bq_sb if ti == 0 else bk_sb
                nc.tensor.matmul(
                    ps,
                    lhsT=bias_sb[0:1, t * 128:(t + 1) * 128],
                    rhs=ones_sb[0:1, 0:512],
                    start=False, stop=True)
            if ti == 1:
                # km carries the key mask AND the 1/sqrt(dk) scale: masked
                # kT columns become exact zeros (kills PE switching power in
                # the scores matmuls of short-length cores) and the exp
                # needs no scale/bias, so one ACT call covers both heads.
                nc.vector.tensor_tensor(
                    out=dstT[:, nch * 512:(nch + 1) * 512], in0=ps,
                    in1=km_sb[:, nch * 512:(nch + 1) * 512],
                    op=mybir.AluOpType.mult)
            else:
                nc.vector.tensor_copy(
                    out=dstT[:, nch * 512:(nch + 1) * 512], in_=ps)

        probs = {}

        def emit_scores_sk(t, sk):
            qT, kT = qk_tiles[t][:2]
            # kT carries the 1/sqrt(dk) scale and the key mask (km), so the
            # exp is pure: no scale or per-partition bias needed.
            pss = []
            for hl in range(2):
                pss.append(sc.tile([128, S], f32, tag="sc",
                                   name=f"sc{t}_{sk}_{hl}"))
            for hl in range(2):
                lo, hi = hl * 64, (hl + 1) * 64
                for nch in range(2):
                    nc.tensor.matmul(
                        pss[hl][:, nch * 512:(nch + 1) * 512],
                        lhsT=kT[lo:hi, sk * 128:(sk + 1) * 128],
                        rhs=qT[lo:hi, nch * 512:(nch + 1) * 512],
                        start=True, stop=True)
            for hl in range(2):
                pb = prpool.tile([128, S], bf16, tag=f"pb{t % 2}_{hl}_{sk}",
                                 name=f"pb{t}_{hl}_{sk}")
                nc.scalar.activation(out=pb, in_=pss[hl], func=EXP,
                                     bias=mb_sb[:, 0:1])
                probs[(t % 2, hl, sk)] = pb

        # ctx batches: per pair, 16 chains (hl, sq) of 65 psum cols each,
        # packed 7 + 7 + 2 into three bank-tiles (tags cxA, cxB, cxA).
        CHAINS = [(hl, sq) for hl in range(2) for sq in range(NST)]
        BATCHES = [(0, 7, "cxA"), (7, 14, "cxB"), (14, 16, "cxA")]

        def emit_ctx_batch(t, bi):
            c0, c1, tag = BATCHES[bi]
            nch = c1 - c0
            pc = cx.tile([128, nch * (DK + 1)], f32, tag=tag,
                         name=f"cx{t}_{bi}")
            for ci in range(nch):
                hl, sq = CHAINS[c0 + ci]
                head = 2 * t + hl
                for sk in range(n_sk):
                    nc.tensor.matmul(
                        pc[:, ci * (DK + 1):ci * (DK + 1) + DK + 1],
                        lhsT=probs[(t % 2, hl, sk)][:, sq * 128:(sq + 1) * 128],
                        rhs=v_sb[:, sk, head * (DK + 1):(head + 1) * (DK + 1)],
                        start=(sk == 0), stop=(sk == n_sk - 1))
            return pc

        def emit_ctx_normalize(t, bi, pc):
            c0, c1, tag = BATCHES[bi]
            nch = c1 - c0
            pc3 = pc.rearrange("p (c x) -> p c x", x=DK + 1)
            rec = smpool.tile([128, nch], f32, name=f"rec{t}_{bi}")
            nc.vector.reciprocal(rec, pc3[:, :, DK])
            # runs of consecutive chains with the same head
            ci = 0
            while ci < nch:
                hl0 = CHAINS[c0 + ci][0]
                cj = ci
                while cj < nch and CHAINS[c0 + cj][0] == hl0:
                    cj += 1
                cnt = cj - ci
                head = 2 * t + hl0
                sq0 = CHAINS[c0 + ci][1]
                out_ap = out_sb[:, sq0:sq0 + cnt, head * DK:(head + 1) * DK]
                in0 = pc3[:, ci:cj, 0:DK]
                in1 = rec[:, ci:cj].unsqueeze(-1).to_broadcast([128, cnt, DK])
                nc.vector.tensor_tensor(out=out_ap, in0=in0, in1=in1, op=MULT)
                ci = cj

        def emit_out_strip(t):
            # pair t's 128 output columns for all 1024 rows, 2 DMAs
            for half, eng in ((0, nc.gpsimd), (1, nc.sync)):
                rows = out_d[half * 512:(half + 1) * 512,
                             t * 128:(t + 1) * 128]
                eng.dma_start(
                    out=rows.rearrange("(s p) c -> p s c", p=128),
                    in_=out_sb[:, half * 4:(half + 1) * 4,
                               t * 128:(t + 1) * 128])

        def emit_out_batch(t, bi, eng):
            # DMA just the chains normalized in batch bi (tail overlap)
            c0, c1, _ = BATCHES[bi]
            ci = c0
            while ci < c1:
                hl0, sq0 = CHAINS[ci]
                cj = ci
                while cj < c1 and CHAINS[cj][0] == hl0:
                    cj += 1
                cnt = cj - ci
                head = 2 * t + hl0
                rows = out_d[sq0 * 128:(sq0 + cnt) * 128,
                             head * DK:(head + 1) * DK]
                eng.dma_start(
                    out=rows.rearrange("(s p) c -> p s c", p=128),
                    in_=out_sb[:, sq0:sq0 + cnt, head * DK:(head + 1) * DK])
                ci = cj

        # ---- main pipeline
        start_qkproj(0)
        for c in range(4):
            emit_qkproj_chunk(0, c)
        prefetch_wqk(1)
        vi = 0
        for sk in range(n_sk):
            emit_scores_sk(0, sk)
            while vi < NST * (sk + 1) // n_sk:
                emit_vproj(vi)
                vi += 1
        while vi < NST:
            emit_vproj(vi)
            vi += 1
        start_qkproj(1)
        for c in range(4):
            emit_qkproj_chunk(1, c)

        for t in range(1, NDT):
            last = (t + 1 >= NDT)
            if not last:
                prefetch_wqk(t + 1)
            pcs = {}

            def run_action(ai, t=t):
                if ai == 0:
                    pcs[0] = emit_ctx_batch(t - 1, 0)
                elif ai == 1:
                    emit_ctx_normalize(t - 1, 0, pcs[0])
                    pcs[1] = emit_ctx_batch(t - 1, 1)
                elif ai == 2:
                    emit_ctx_normalize(t - 1, 1, pcs[1])
                    pcs[2] = emit_ctx_batch(t - 1, 2)
                else:
                    emit_ctx_normalize(t - 1, 2, pcs[2])
                    emit_out_strip(t - 1)

            thresholds = [3, 4, 5, 6] if last else [4, 5, 6, 7]
            ai = 0
            for sk in range(n_sk):
                emit_scores_sk(t, sk)
                if not last and sk < 4:
                    if sk == 0:
                        start_qkproj(t + 1)
                    emit_qkproj_chunk(t + 1, sk)
                while ai < 4 and sk >= min(thresholds[ai], n_sk - 1):
                    run_action(ai)
                    ai += 1
            if not last:
                for c in range(min(4, n_sk), 4):
                    emit_qkproj_chunk(t + 1, c)
            while ai < 4:
                run_action(ai)
                ai += 1

        # epilogue: ctx of the last pair; DMA each batch as it normalizes
        t = NDT - 1
        pc0 = emit_ctx_batch(t, 0)
        pc1 = emit_ctx_batch(t, 1)
        emit_ctx_normalize(t, 0, pc0)
        emit_out_batch(t, 0, nc.gpsimd)
        pc2 = emit_ctx_batch(t, 2)
        emit_ctx_normalize(t, 1, pc1)
        emit_out_batch(t, 1, nc.sync)
        emit_ctx_normalize(t, 2, pc2)
        emit_out_batch(t, 2, nc.gpsimd)

    return nc


TRACE = False
LAST_EXEC_NS = None
LAST_RES = None


def kernel(Q, length, Wq, bq, Wk, bk, Wv, bv):
    global LAST_EXEC_NS, LAST_RES
    _install_shims()
    from concourse.bass_utils import run_bass_kernel_spmd

    Q = np.asarray(Q, np.float32)
    length = np.asarray(length, np.int32)
    Wq, Wk, Wv = (np.asarray(w, np.float32) for w in (Wq, Wk, Wv))
    bq, bk, bv = (np.asarray(b, np.float32) for b in (bq, bk, bv))

    use_bias = bool(np.any(bq) or np.any(bk) or np.any(bv))
    maxlen = int(length.max()) if length.size else S
    n_sk = max(1, min(NST, -(-max(1, maxlen) // 128)))

    import ml_dtypes
    bfl = ml_dtypes.bfloat16
    qt_all = np.ascontiguousarray(Q.transpose(0, 2, 1)).astype(bfl)   # [B, 768, 1024]
    wqm = np.ascontiguousarray(Wq.T.reshape(IN_DIM, NDT, 128).transpose(1, 0, 2)).astype(bfl)
    wkm = np.ascontiguousarray(Wk.T.reshape(IN_DIM, NDT, 128).transpose(1, 0, 2)).astype(bfl)
    wvt = np.ascontiguousarray(Wv.T).astype(bfl)                      # [768, 768]
    j = np.arange(S)
    valid = j[None, :] < length[:, None]                                   # [B, S]
    # mb half is the (zero) exp bias AP; mv half is the 0/1 key-row mask
    mb = np.zeros((B, 128, NST), np.float32)
    mv = valid.astype(np.float32).reshape(B, NST, 128).transpose(0, 2, 1)
    mbv = np.ascontiguousarray(np.concatenate([mb, mv], axis=2))           # [B,128,16]
    # km folds the key mask and the 1/sqrt(dk) scale into kT
    kmrow = np.where(valid, np.float32(1.0 / np.sqrt(DK)), np.float32(0.0))
    km = np.ascontiguousarray(
        np.broadcast_to(kmrow.astype(bfl)[:, None, :], (B, 128, S)))       # [B,128,S]

    nc = _build_program(n_sk, use_bias)
    in_maps = []
    for b in range(B):
        m = {"qt": qt_all[b], "wqm": wqm, "wkm": wkm, "wvt": wvt,
             "mbv": mbv[b], "km": km[b]}
        if use_bias:
            m["bq"] = bq.reshape(1, -1).astype(np.float32).astype(bfl)
            m["bk"] = bk.reshape(1, -1).astype(np.float32).astype(bfl)
            m["bv"] = bv.reshape(1, -1).astype(np.float32).astype(bfl)
        in_maps.append(m)

    res = run_bass_kernel_spmd(
        nc, in_maps, core_ids=list(range(NCORES)), trace=TRACE)
    LAST_EXEC_NS = res.exec_time_ns
    LAST_RES = res
    out = np.stack([res.results[b]["out"] for b in range(B)])
    out = np.ascontiguousarray(out.astype(np.float32))
    # reference: attn = p / (sum + 1e-8); for length==0 every key is masked
    # and the reference output is ~0, while our reciprocal normalization
    # averages the tiny masked probs. Zero those rows host-side.
    for b in range(B):
        if int(length[b]) == 0:
            out[b] = 0.0
    return out
